# revision 54
# baseline (speedup 1.0000x reference)
"""Multi-head attention (B=2, S=2048, D=1024, H=16) on 8 trn2 NeuronCores.

Sharding: batch (2) x head-groups (4 heads each, 4 groups) = 8 cores.
Each core computes Q/K/V projections for its 4 heads on its batch,
causal-masked softmax attention, and a partial output projection
(row-sharded w_o); the host sums the 4 partials per batch.

Layout strategy: the host stages every DRAM input in the exact SBUF
layout (partition-major, strip-major x) so each DMA is 128 fully
contiguous descriptors. All matmul contractions run over the SBUF
partition axis with no on-device transposes. Attention scores are
computed transposed (ST[k, q]) so P = exp(ST) feeds the PV matmul
directly, and V carries an appended ones-column (M=65 stationary)
whose output row is the softmax denominator.

v2 scheduling: a burst of dummy matmuls at t=0 holds the PE's HAM
clock-gate open through the initial input DMA; softmax normalization
uses ACT evacuation + reciprocal_approx_fast + a GpSimd partition
broadcast (instead of serial DVE reciprocal/stream-shuffle chains) so
strip boundaries no longer stall the PE; PV accumulation orders the
widest causal window first so no PSUM memsets are needed.
"""
import sys
from contextlib import nullcontext

sys.path.insert(0, "/opt/trn_rl_repo")

import numpy as np
import ml_dtypes

import concourse.bass as bass
import concourse.mybir as mybir
import concourse.tile as tile
from concourse.bass_utils import run_bass_kernel_spmd

B, S, D, H, DK = 2, 2048, 1024, 16, 64
NCORES = 8
HG = 4                # heads per core
DHG = HG * DK         # 256 head-dims per core
KT = D // 128         # 8 contraction tiles for the projections
ST128 = S // 128      # 16 128-row tiles of S
QS = 512              # q-strip width
NQS = S // QS         # 4 strips
VW = 68               # v_sb inner stride (65 used, padded for alignment)
N_WARM = 12           # dummy matmuls: enough to warm the HAM clock-gate
                      # (~5us busy) without outliving the first input DMAs
N_PF1 = 8             # strip-1 score/exp units prefetched into phase A
PB_BUFS = 26          # p-tile ring (holds the prefetched exp outputs)

f32 = mybir.dt.float32
bf16 = mybir.dt.bfloat16
EXP = mybir.ActivationFunctionType.Exp
RECIP = mybir.ActivationFunctionType.Reciprocal


def raw_act(nc, out, in_, func, bias=0.0, scale=1.0):
    """InstActivation without the bass wrapper. Needed for Reciprocal,
    which the wrapper refuses; measured on this hardware it is ~1e-5
    relative error over the softmax-denominator range, far inside our
    tolerance, and one ACT op replaces a 3.3us single-lane DVE
    reciprocal on the strip-end critical path."""
    eng = nc.scalar
    inputs = [
        eng.lower_ap(in_),
        mybir.ImmediateValue(dtype=f32, value=bias),
        mybir.ImmediateValue(dtype=f32, value=scale),
        mybir.ImmediateValue(dtype=f32, value=0.0),
    ]
    return eng.add_instruction(
        mybir.InstActivation(
            name=nc.get_next_instruction_name(),
            func=func,
            ins=inputs,
            outs=[eng.lower_ap(out)],
        )
    )


def _split_waits(nc, max_waits=1):
    """This walrus build rejects >1 SyncWait per instruction (and >0 on
    fp32-family matmuls, which lower through the 1-wait S3_LW struct).
    Hoist excess waits onto dedicated NOPs on the same engine queue."""
    n = 0
    for fn in nc.m.functions:
        for blk in fn.blocks:
            new = []
            for ins in blk.instructions:
                si = getattr(ins, "sync_info", None)
                if si is not None and si.on_wait:
                    limit = 0 if isinstance(ins, mybir.InstMatmult) else max_waits
                    if len(si.on_wait) > limit:
                        waits = list(si.on_wait)
                        hoist = waits if limit == 0 else waits[:-limit]
                        keep = [] if limit == 0 else waits[-limit:]
                        for w in hoist:
                            n += 1
                            new.append(
                                mybir.InstNoOp(
                                    name=f"I-waitfix-{n}",
                                    engine=ins.engine,
                                    bass_nofuse=True,
                                    sync_info=mybir.SyncInfo(
                                        on_wait=[w], on_update=[]
                                    ),
                                )
                            )
                        ins.sync_info = mybir.SyncInfo(
                            on_wait=keep, on_update=list(si.on_update)
                        )
                new.append(ins)
            blk.instructions[:] = new
    return n


def classify_mask(maskT):
    """Block-classify the transposed mask at 128x128 granularity.
    Returns (cls[i,j] in {0 empty,1 full,2 partial}, bias index map,
    list of multiplicative {0,1} blocks for the partial ones)."""
    nb = S // 128
    cls = np.empty((nb, nb), dtype=np.int8)
    bidx = np.full((nb, nb), -1, dtype=np.int32)
    biases = []
    for i in range(nb):
        for j in range(nb):
            blk = maskT[i * 128 : (i + 1) * 128, j * 128 : (j + 1) * 128]
            if (blk != 0).all():
                cls[i, j] = 1
            elif (blk == 0).all():
                cls[i, j] = 0
            else:
                cls[i, j] = 2
                bidx[i, j] = len(biases)
                biases.append(
                    (blk != 0).astype(np.float32)
                )
    return cls, bidx, biases


def build_program(cls, bidx, n_bias):
    nb_alloc = max(1, n_bias)
    nc = bass.Bass("TRN2", target_bir_lowering=False, debug=False,
                   num_devices=NCORES)
    xq_d = nc.dram_tensor("xqT", [NQS, 128, KT, QS], bf16,
                          kind="ExternalInput").ap()
    xk_d = nc.dram_tensor("xkT", [NQS, 128, KT, QS], bf16,
                          kind="ExternalInput").ap()
    xv_d = nc.dram_tensor("xvT", [NQS, 128, KT, QS], bf16,
                          kind="ExternalInput").ap()
    wq_d = nc.dram_tensor("wqT", [128, KT, DHG], bf16,
                          kind="ExternalInput").ap()
    wk_d = nc.dram_tensor("wkT", [128, KT, DHG], bf16,
                          kind="ExternalInput").ap()
    wv_d = nc.dram_tensor("wvT", [128, KT, DHG], bf16,
                          kind="ExternalInput").ap()
    wo_d = nc.dram_tensor("woT", [128, 2, D], bf16, kind="ExternalInput").ap()
    bias_d = nc.dram_tensor("biasT", [128, nb_alloc, 128], bf16,
                            kind="ExternalInput").ap()
    y_d = nc.dram_tensor("y", [S, D], bf16, kind="ExternalOutput").ap()

    # per-strip causal-window metadata, shared by the prefetch and the
    # strip loop. kts order: a widest-window tile first (its PV opens the
    # PSUM accumulation and must cover every later window), then partial
    # (masked) tiles so their DVE mask-multiplies run early, then the rest.
    strips = []
    for qs in range(NQS):
        sub_all = cls[:, 4 * qs : 4 * qs + 4]
        kts = [i for i in range(ST128) if sub_all[i].any()]

        def window(kt):
            nz = np.nonzero(sub_all[kt])[0]
            return int(nz.min()) * 128

        c0_min = min((window(kt) for kt in kts), default=0)
        parts = sorted(
            (kt for kt in kts if (sub_all[kt] == 2).any()), key=window
        )
        fulls = sorted(
            (kt for kt in kts if (sub_all[kt] != 2).all()), key=window
        )
        cand = [kt for kt in fulls if window(kt) == c0_min] or [
            kt for kt in parts if window(kt) == c0_min
        ]
        kts = [cand[0]] + [kt for kt in parts + fulls if kt != cand[0]]
        strips.append((kts, c0_min, sub_all))

    with tile.TileContext(nc) as tc:
        with tc.tile_pool(name="persist", bufs=1) as pp, tc.tile_pool(
            name="pb", bufs=PB_BUFS
        ) as pb, tc.tile_pool(
            name="otu", bufs=2
        ) as otup, tc.tile_pool(
            name="yp", bufs=3
        ) as yp, tc.tile_pool(
            name="psS", bufs=2, space="PSUM"
        ) as psS, tc.tile_pool(
            name="xw", bufs=2
        ) as xw:
            qt_sb = pp.tile([128, 2, S], bf16)             # Q^T head pairs
            ktp_sb = pp.tile([128, HG, S], bf16)           # K^T padded/head
            v_sb = pp.tile([128, ST128, HG, VW], bf16)     # V | ones col
            ot_sb = pp.tile([128, 2, S], bf16)             # attn out^T
            wo_sb = pp.tile([128, 2, D], bf16)
            bias_sb = pp.tile([128, nb_alloc, 128], bf16)
            ones_sb = pp.tile([128, 128], bf16)            # bc stationary
            recb_sb = pp.tile([128, HG, QS], bf16)         # 1/den rows (@64)

            nc.gpsimd.memset(ones_sb[:], 1.0)
            nc.gpsimd.memset(
                recb_sb[:].rearrange("p a b -> p (a b)"), 0.0
            )
            nc.vector.memset(
                ktp_sb[:].rearrange("p a b -> p (a b)"), 0.0
            )
            nc.vector.memset(
                v_sb[:, :, :, 64:65].rearrange("p a b c -> p (a b c)"), 1.0
            )

            prefetched = {}

            def emit_scores_exp(qs, kt):
                kts, c0_min, sub_all = strips[qs]
                sub = sub_all[kt]
                nz = np.nonzero(sub)[0]
                c0 = int(nz.min()) * 128
                c1 = (int(nz.max()) + 1) * 128
                partial_js = [j for j in range(4) if sub[j] == 2]
                interior = [
                    j for j in range(4)
                    if sub[j] == 0 and c0 // 128 < j < c1 // 128
                ]
                pair_ps = {}
                for mt in range(2):
                    ps = psS.tile(
                        [128, 2, QS], f32, tag="ps", name=f"pp{mt}"
                    )
                    pair_ps[mt] = ps
                    for hh in range(2):
                        h = 2 * mt + hh
                        nc.tensor.matmul(
                            ps[:, hh, c0:c1],
                            ktp_sb[:, h, kt * 128 : (kt + 1) * 128],
                            qt_sb[:, mt, qs * QS + c0 : qs * QS + c1],
                            start=True,
                            stop=True,
                        )
                pair_p = {}
                for mt in range(2):
                    p_sb = pb.tile(
                        [128, 2, QS], bf16, tag="p", name=f"p{mt}"
                    )
                    pair_p[mt] = p_sb
                    for j in interior:
                        for hh in range(2):
                            nc.vector.memset(
                                p_sb[:, hh, j * 128 : (j + 1) * 128], 0.0
                            )
                    nc.scalar.activation(
                        p_sb[:, :, c0:c1],
                        pair_ps[mt][:, :, c0:c1],
                        EXP,
                        scale=0.125,
                    )
                    for j in partial_js:
                        bi = int(bidx[kt, 4 * qs + j])
                        for hh in range(2):
                            nc.vector.tensor_mul(
                                p_sb[:, hh, j * 128 : (j + 1) * 128],
                                p_sb[:, hh, j * 128 : (j + 1) * 128],
                                bias_sb[:, bi, :],
                            )
                return pair_p, c0

            # scores+exp of the first strips are emitted inside phase A,
            # interleaved with the projection chains: the ACT engine (the
            # strip phase pacer) starts its exp stream ~30us early.
            pf_units = [(0, kt) for kt in strips[0][0]]
            pf_units += [(1, kt) for kt in strips[1][0]][:N_PF1]
            pf_it = iter(list(pf_units))

            def pump_units(n):
                for _ in range(n):
                    u = next(pf_it, None)
                    if u is None:
                        return
                    prefetched[u] = emit_scores_exp(*u)

            # ---- Phase A: warmup + projections + prefetch ----
            deferred_v = []
            with tc.tile_pool(name="psA", bufs=4, space="PSUM") as psA:
                # Dummy matmuls: keep the PE active (and the HAM clock
                # un-throttled) while the first inputs stream in. They read
                # qt_sb before anything writes it — garbage values into a
                # scratch PSUM slot that is never read, zero dependencies.
                wps = psS.tile([128, 2, QS], f32, tag="ps", name="warm")
                for _ in range(N_WARM):
                    nc.tensor.matmul(
                        wps[:, 0, :], qt_sb[:, 0, 0:128], qt_sb[:, 0, 0:512],
                        start=True, stop=True,
                    )
                for x_d, w_d, which in (
                    (xq_d, wq_d, "q"),
                    (xk_d, wk_d, "k"),
                    (xv_d, wv_d, "v"),
                ):
                    xt = xw.tile([128, NQS, KT, QS], bf16, tag="xT")
                    wt = xw.tile([128, KT, DHG], bf16, tag="wT")
                    nc.sync.dma_start(out=wt[:], in_=w_d)
                    for qs in range(NQS):
                        nc.sync.dma_start(out=xt[:, qs], in_=x_d[qs])
                    if which == "q":
                        # queued behind the q inputs: needed only from the
                        # first strip's exp / y-projection onward
                        nc.sync.dma_start(out=wo_sb[:], in_=wo_d)
                        if n_bias:
                            nc.sync.dma_start(out=bias_sb[:], in_=bias_d)
                    if which in ("q", "k"):
                        for qs in range(NQS):
                            for mt in range(2):
                                ps = psA.tile([128, QS], f32, tag="pA")
                                for kt in range(KT):
                                    nc.tensor.matmul(
                                        ps[:],
                                        wt[:, kt, mt * 128 : (mt + 1) * 128],
                                        xt[:, qs, kt, :],
                                        start=(kt == 0),
                                        stop=(kt == KT - 1),
                                    )
                                if which == "q":
                                    nc.vector.tensor_copy(
                                        out=qt_sb[
                                            :, mt, qs * QS : (qs + 1) * QS
                                        ],
                                        in_=ps[:],
                                    )
                                else:
                                    # per-head K-padded keys: head at its
                                    # own partition range, others stay zero
                                    for hh in range(2):
                                        h = 2 * mt + hh
                                        po = 64 * hh
                                        nc.vector.tensor_copy(
                                            out=ktp_sb[
                                                po : po + 64, h,
                                                qs * QS : (qs + 1) * QS,
                                            ],
                                            in_=ps[po : po + 64, :],
                                        )
                            if which == "k" and qs <= 1:
                                pump_units(2)
                    else:
                        def make_vchain(st, xt=xt, wt=wt):
                            # deferred V chain: runs from a psS slot during
                            # the s0->s1 normalize stall — the only strip
                            # needing these key tiles is the last one
                            def f():
                                qv, sc = divmod(st, NQS)
                                ps = psS.tile([128, 2, QS], f32, tag="ps",
                                              name=f"vd{st}")
                                for kt in range(KT):
                                    nc.tensor.matmul(
                                        ps[:, 0, :DHG],
                                        xt[:, qv, kt,
                                           sc * 128 : (sc + 1) * 128],
                                        wt[:, kt, :],
                                        start=(kt == 0),
                                        stop=(kt == KT - 1),
                                    )
                                nc.vector.tensor_copy(
                                    out=v_sb[:, st, :, 0:DK],
                                    in_=ps[:, 0, :DHG].rearrange(
                                        "p (h d) -> p h d", h=HG
                                    ),
                                )
                            return f

                        for st in range(ST128):
                            if st >= ST128 - 2:
                                deferred_v.append(make_vchain(st))
                                continue
                            qs, sc = divmod(st, NQS)
                            ps = psA.tile([128, QS], f32, tag="pA")
                            for kt in range(KT):
                                nc.tensor.matmul(
                                    ps[:, :DHG],
                                    xt[:, qs, kt, sc * 128 : (sc + 1) * 128],
                                    wt[:, kt, :],
                                    start=(kt == 0),
                                    stop=(kt == KT - 1),
                                )
                            nc.vector.tensor_copy(
                                out=v_sb[:, st, :, 0:DK],
                                in_=ps[:, :DHG].rearrange(
                                    "p (h d) -> p h d", h=HG
                                ),
                            )
                            pump_units(1)
                pump_units(len(pf_units))

            # ---- Phases B+C interleaved per q-strip ----
            # recip-gated boundary instructions get LOW priority (large
            # value) so the list scheduler statically orders any ready
            # filler work ahead of them; dependency edges still pin their
            # execution, so this is order-only, correctness-neutral
            deprio = [1 << 20]

            def lower(mm):
                mm.bass_priority = deprio[0]
                deprio[0] += 1

            with tc.tile_pool(name="psOT", bufs=1, space="PSUM") as psOT:

                def emit_yproj(st, hi=False):
                    ps = psS.tile(
                        [128, 2, QS], f32, tag="ps", name=f"py{st}"
                    )
                    # hi: boundary-filler y-projections — schedule their
                    # matmuls ahead of the recip-gated bc/PV ops (the psS
                    # ring still gates them late enough not to hoist)
                    ctx = tc.high_priority() if hi else nullcontext()
                    with ctx:
                        for nh in range(2):
                            for mt in range(2):
                                nc.tensor.matmul(
                                    ps[:, nh, :],
                                    ot_sb[:, mt, st * 128 : (st + 1) * 128],
                                    wo_sb[:, mt, nh * QS : (nh + 1) * QS],
                                    start=(mt == 0),
                                    stop=(mt == 1),
                                )
                    y_sb = yp.tile([128, 2, QS], bf16, tag="y",
                                   name=f"ysb{st}")
                    nc.vector.tensor_copy(
                        out=y_sb[:].rearrange("p a b -> p (a b)"),
                        in_=ps[:].rearrange("p a b -> p (a b)"),
                    )
                    nc.sync.dma_start(
                        out=y_d[st * 128 : (st + 1) * 128, :],
                        in_=y_sb[:].rearrange("p a b -> p (a b)"),
                    )

                norm_pending = None
                pump_idx = {1: N_PF1}
                for qs in range(NQS):
                    kts, c0_min, sub_all = strips[qs]
                    pend_y = (
                        [(qs - 1) * (QS // 128) + i for i in range(QS // 128)]
                        if qs
                        else []
                    )

                    def alloc_pots():
                        # one tile, 4 banks: head h in column block h, its
                        # softmax denominator on partition row 64 — so the
                        # 4 denominator rows form a single AP and the strip
                        # reciprocal is ONE unsplittable ACT instruction
                        pots = psOT.tile(
                            [128, HG, QS], f32, tag="pot", name="pots"
                        )
                        if c0_min > 0:
                            nc.vector.memset(
                                pots[0:65, :, :].rearrange(
                                    "p a b -> p (a b)"
                                ),
                                0.0,
                            )
                        return pots

                    pots = None if norm_pending else alloc_pots()
                    deferred_pv = []
                    for idx, kt in enumerate(kts):
                        was_pf = (qs, kt) in prefetched
                        if was_pf:
                            pair_p, c0 = prefetched.pop((qs, kt))
                        else:
                            pair_p, c0 = emit_scores_exp(qs, kt)
                        # cascade: while this strip rides on prefetched
                        # exps (ACT slack), pre-emit the next strip's first
                        # score/exp units so its boundary hides the same way
                        if was_pf and qs + 1 < NQS:
                            nkts = strips[qs + 1][0]
                            j = pump_idx.get(qs + 1, 0)
                            if j < min(len(nkts), N_PF1):
                                pump_idx[qs + 1] = j + 1
                                prefetched[(qs + 1, nkts[j])] = (
                                    emit_scores_exp(qs + 1, nkts[j])
                                )
                        if norm_pending is not None and idx == 1:
                            # ungated PE filler for the normalize funnel:
                            # the deferred V chains depend only on resident
                            # phase-A data
                            while deferred_v:
                                deferred_v.pop(0)()
                            norm_pending()
                            norm_pending = None
                            pots = alloc_pots()

                        def emit_pv(lo=False, kt=kt, idx=idx, c0=c0,
                                    pair_p=pair_p):
                            for h in range(HG):
                                mm = nc.tensor.matmul(
                                    pots[0:65, h, c0:],
                                    v_sb[:, kt, h, 0:65],
                                    pair_p[h // 2][:, h % 2, c0:],
                                    start=(idx == 0 and c0_min == 0),
                                    stop=(idx == len(kts) - 1),
                                    skip_group_check=(c0_min > 0),
                                )
                                if lo:
                                    lower(mm)

                        if pots is None:
                            deferred_pv.append(emit_pv)
                        else:
                            while deferred_pv:
                                deferred_pv.pop(0)(True)
                            emit_pv()
                        # hold the last 2 back: they read the PREVIOUS
                        # strip's (long-finished) output, so they are
                        # ungated filler for the strip-end normalize funnel
                        if len(pend_y) > 2 and idx >= 3:
                            emit_yproj(pend_y.pop(0))

                    def make_norm(qs=qs, pots=pots):
                        def norm():
                            # one recip(ACT) + PE ones-broadcast + DVE scale
                            otu = otup.tile([128, HG, QS], f32, tag="otu")
                            with tc.high_priority():
                                raw_act(
                                    nc, recb_sb[64:65, :, :],
                                    pots[64:65, :, :], RECIP,
                                )
                                nc.vector.tensor_copy(
                                    out=otu[0:64, :, :].rearrange(
                                        "p a b -> p (a b)"
                                    ),
                                    in_=pots[0:64, :, :].rearrange(
                                        "p a b -> p (a b)"
                                    ),
                                )
                            bct = {}
                            for h in range(HG):
                                po = 64 * (h % 2)
                                mt = h // 2
                                hh = h % 2
                                if hh == 0:
                                    bct[mt] = psS.tile(
                                        [128, 2, QS], f32, tag="ps",
                                        name=f"bc{mt}",
                                    )
                                lower(nc.tensor.matmul(
                                    bct[mt][:, hh, :],
                                    ones_sb[:],
                                    recb_sb[:, h, :],
                                    start=True,
                                    stop=True,
                                ))
                                nc.vector.tensor_mul(
                                    ot_sb[
                                        po : po + 64, mt,
                                        qs * QS : (qs + 1) * QS,
                                    ],
                                    otu[0:64, h, :],
                                    bct[mt][0:64, hh, :],
                                )
                        return norm

                    for st in pend_y:
                        emit_yproj(st, hi=True)
                    norm_pending = make_norm()
                    if qs == NQS - 1:
                        norm_pending()
                        norm_pending = None
                        for sti in range(QS // 128):
                            emit_yproj(qs * (QS // 128) + sti)

    _split_waits(nc)
    return nc


_program_cache = {}


def get_program(cls, bidx, n_bias):
    key = (cls.tobytes(), bidx.tobytes(), n_bias)
    if key not in _program_cache:
        _program_cache[key] = build_program(cls, bidx, n_bias)
    return _program_cache[key]


def _x_layout(xb):
    """x[b].T as [NQS, 128, KT, QS] bf16 (strip-major, partition-major)."""
    xt = np.asarray(xb, np.float32).T                       # [D, S]
    arr = xt.reshape(KT, 128, NQS, QS).transpose(2, 1, 0, 3)
    return np.ascontiguousarray(arr).astype(ml_dtypes.bfloat16)


def _w_layout(w, rows):
    """w[rows].T as [128, KT, DHG] bf16."""
    wt = np.asarray(w, np.float32)[rows].T                  # [D, DHG]
    arr = wt.reshape(KT, 128, DHG).transpose(1, 0, 2)
    return np.ascontiguousarray(arr).astype(ml_dtypes.bfloat16)


def _wo_layout(w_o, rows):
    """w_o[:, rows].T as [128, 2, D] bf16."""
    wt = np.asarray(w_o, np.float32)[:, rows].T             # [DHG, D]
    arr = wt.reshape(2, 128, D).transpose(1, 0, 2)
    return np.ascontiguousarray(arr).astype(ml_dtypes.bfloat16)


def make_in_maps(q, k, v, mask, w_q, w_k, w_v, w_o, biases):
    if biases:
        bias_arr = np.ascontiguousarray(
            np.stack(biases).transpose(1, 0, 2)
        ).astype(ml_dtypes.bfloat16)
    else:
        bias_arr = np.zeros((128, 1, 128), ml_dtypes.bfloat16)
    xs = [
        {"xqT": _x_layout(q[b]), "xkT": _x_layout(k[b]),
         "xvT": _x_layout(v[b])}
        for b in range(B)
    ]
    ws = []
    for g in range(4):
        rows = slice(g * DHG, (g + 1) * DHG)
        ws.append(
            {
                "wqT": _w_layout(w_q, rows),
                "wkT": _w_layout(w_k, rows),
                "wvT": _w_layout(w_v, rows),
                "woT": _wo_layout(w_o, rows),
            }
        )
    in_maps = []
    for c in range(NCORES):
        b, g = divmod(c, 4)
        in_maps.append({**xs[b], **ws[g], "biasT": bias_arr})
    return in_maps


def combine_results(results):
    out = np.empty((B, S, D), np.float32)
    for b in range(B):
        acc = results[4 * b]["y"].astype(np.float32)
        for g in range(1, 4):
            acc = acc + results[4 * b + g]["y"].astype(np.float32)
        out[b] = acc
    return out


def kernel(q, k, v, mask, w_q, w_k, w_v, w_o):
    q = np.asarray(q, np.float32)
    k = np.asarray(k, np.float32)
    v = np.asarray(v, np.float32)
    w_q = np.asarray(w_q, np.float32)
    w_k = np.asarray(w_k, np.float32)
    w_v = np.asarray(w_v, np.float32)
    w_o = np.asarray(w_o, np.float32)
    maskT = np.ascontiguousarray(
        np.broadcast_to(np.asarray(mask), (1, 1, S, S))[0, 0].T
    )
    cls, bidx, biases = classify_mask(maskT)
    nc = get_program(cls, bidx, len(biases))
    in_maps = make_in_maps(q, k, v, mask, w_q, w_k, w_v, w_o, biases)
    res = run_bass_kernel_spmd(nc, in_maps, list(range(NCORES)))
    return combine_results(res.results)


# revision 55
# speedup vs baseline: 1.0208x; 1.0208x over previous
"""Multi-head attention (B=2, S=2048, D=1024, H=16) on 8 trn2 NeuronCores.

Sharding: batch (2) x head-groups (4 heads each, 4 groups) = 8 cores.
Each core computes Q/K/V projections for its 4 heads on its batch,
causal-masked softmax attention, and a partial output projection
(row-sharded w_o); the host sums the 4 partials per batch.

Layout strategy: the host stages every DRAM input in the exact SBUF
layout (partition-major, strip-major x) so each DMA is 128 fully
contiguous descriptors. All matmul contractions run over the SBUF
partition axis with no on-device transposes. Attention scores are
computed transposed (ST[k, q]) so P = exp(ST) feeds the PV matmul
directly, and V carries an appended ones-column (M=65 stationary)
whose output row is the softmax denominator.

v2 scheduling: a burst of dummy matmuls at t=0 holds the PE's HAM
clock-gate open through the initial input DMA; softmax normalization
uses ACT evacuation + reciprocal_approx_fast + a GpSimd partition
broadcast (instead of serial DVE reciprocal/stream-shuffle chains) so
strip boundaries no longer stall the PE; PV accumulation orders the
widest causal window first so no PSUM memsets are needed.
"""
import sys
from contextlib import nullcontext

sys.path.insert(0, "/opt/trn_rl_repo")

import numpy as np
import ml_dtypes

import concourse.bass as bass
import concourse.mybir as mybir
import concourse.tile as tile
from concourse.bass_utils import run_bass_kernel_spmd

B, S, D, H, DK = 2, 2048, 1024, 16, 64
NCORES = 8
HG = 4                # heads per core
DHG = HG * DK         # 256 head-dims per core
KT = D // 128         # 8 contraction tiles for the projections
ST128 = S // 128      # 16 128-row tiles of S
QS = 512              # q-strip width
NQS = S // QS         # 4 strips
VW = 68               # v_sb inner stride (65 used, padded for alignment)
N_WARM = 16           # dummy matmuls to hold the HAM clock-gate open
N_PF1 = 8             # strip-1 score/exp units prefetched into phase A
PB_BUFS = 26          # p-tile ring (holds the prefetched exp outputs)

f32 = mybir.dt.float32
bf16 = mybir.dt.bfloat16
EXP = mybir.ActivationFunctionType.Exp
RECIP = mybir.ActivationFunctionType.Reciprocal


def raw_act(nc, out, in_, func, bias=0.0, scale=1.0):
    """InstActivation without the bass wrapper. Needed for Reciprocal,
    which the wrapper refuses; measured on this hardware it is ~1e-5
    relative error over the softmax-denominator range, far inside our
    tolerance, and one ACT op replaces a 3.3us single-lane DVE
    reciprocal on the strip-end critical path."""
    eng = nc.scalar
    inputs = [
        eng.lower_ap(in_),
        mybir.ImmediateValue(dtype=f32, value=bias),
        mybir.ImmediateValue(dtype=f32, value=scale),
        mybir.ImmediateValue(dtype=f32, value=0.0),
    ]
    return eng.add_instruction(
        mybir.InstActivation(
            name=nc.get_next_instruction_name(),
            func=func,
            ins=inputs,
            outs=[eng.lower_ap(out)],
        )
    )


def _split_waits(nc, max_waits=1):
    """This walrus build rejects >1 SyncWait per instruction (and >0 on
    fp32-family matmuls, which lower through the 1-wait S3_LW struct).
    Hoist excess waits onto dedicated NOPs on the same engine queue."""
    n = 0
    for fn in nc.m.functions:
        for blk in fn.blocks:
            new = []
            for ins in blk.instructions:
                si = getattr(ins, "sync_info", None)
                if si is not None and si.on_wait:
                    limit = 0 if isinstance(ins, mybir.InstMatmult) else max_waits
                    if len(si.on_wait) > limit:
                        waits = list(si.on_wait)
                        hoist = waits if limit == 0 else waits[:-limit]
                        keep = [] if limit == 0 else waits[-limit:]
                        for w in hoist:
                            n += 1
                            new.append(
                                mybir.InstNoOp(
                                    name=f"I-waitfix-{n}",
                                    engine=ins.engine,
                                    bass_nofuse=True,
                                    sync_info=mybir.SyncInfo(
                                        on_wait=[w], on_update=[]
                                    ),
                                )
                            )
                        ins.sync_info = mybir.SyncInfo(
                            on_wait=keep, on_update=list(si.on_update)
                        )
                new.append(ins)
            blk.instructions[:] = new
    return n


def classify_mask(maskT):
    """Block-classify the transposed mask at 128x128 granularity.
    Returns (cls[i,j] in {0 empty,1 full,2 partial}, bias index map,
    list of multiplicative {0,1} blocks for the partial ones)."""
    nb = S // 128
    cls = np.empty((nb, nb), dtype=np.int8)
    bidx = np.full((nb, nb), -1, dtype=np.int32)
    biases = []
    for i in range(nb):
        for j in range(nb):
            blk = maskT[i * 128 : (i + 1) * 128, j * 128 : (j + 1) * 128]
            if (blk != 0).all():
                cls[i, j] = 1
            elif (blk == 0).all():
                cls[i, j] = 0
            else:
                cls[i, j] = 2
                bidx[i, j] = len(biases)
                biases.append(
                    (blk != 0).astype(np.float32)
                )
    return cls, bidx, biases


def build_program(cls, bidx, n_bias):
    nb_alloc = max(1, n_bias)
    nc = bass.Bass("TRN2", target_bir_lowering=False, debug=False,
                   num_devices=NCORES)
    xq_d = nc.dram_tensor("xqT", [NQS, 128, KT, QS], bf16,
                          kind="ExternalInput").ap()
    xk_d = nc.dram_tensor("xkT", [NQS, 128, KT, QS], bf16,
                          kind="ExternalInput").ap()
    xv_d = nc.dram_tensor("xvT", [NQS, 128, KT, QS], bf16,
                          kind="ExternalInput").ap()
    wq_d = nc.dram_tensor("wqT", [128, KT, DHG], bf16,
                          kind="ExternalInput").ap()
    wk_d = nc.dram_tensor("wkT", [128, KT, DHG], bf16,
                          kind="ExternalInput").ap()
    wv_d = nc.dram_tensor("wvT", [128, KT, DHG], bf16,
                          kind="ExternalInput").ap()
    wo_d = nc.dram_tensor("woT", [128, 2, D], bf16, kind="ExternalInput").ap()
    bias_d = nc.dram_tensor("biasT", [128, nb_alloc, 128], bf16,
                            kind="ExternalInput").ap()
    y_d = nc.dram_tensor("y", [S, D], bf16, kind="ExternalOutput").ap()

    # per-strip causal-window metadata, shared by the prefetch and the
    # strip loop. kts order: a widest-window tile first (its PV opens the
    # PSUM accumulation and must cover every later window), then partial
    # (masked) tiles so their DVE mask-multiplies run early, then the rest.
    strips = []
    for qs in range(NQS):
        sub_all = cls[:, 4 * qs : 4 * qs + 4]
        kts = [i for i in range(ST128) if sub_all[i].any()]

        def window(kt):
            nz = np.nonzero(sub_all[kt])[0]
            return int(nz.min()) * 128

        c0_min = min((window(kt) for kt in kts), default=0)
        parts = sorted(
            (kt for kt in kts if (sub_all[kt] == 2).any()), key=window
        )
        fulls = sorted(
            (kt for kt in kts if (sub_all[kt] != 2).all()), key=window
        )
        cand = [kt for kt in fulls if window(kt) == c0_min] or [
            kt for kt in parts if window(kt) == c0_min
        ]
        kts = [cand[0]] + [kt for kt in parts + fulls if kt != cand[0]]
        strips.append((kts, c0_min, sub_all))

    with tile.TileContext(nc) as tc:
        with tc.tile_pool(name="persist", bufs=1) as pp, tc.tile_pool(
            name="pb", bufs=PB_BUFS
        ) as pb, tc.tile_pool(
            name="otu", bufs=2
        ) as otup, tc.tile_pool(
            name="yp", bufs=3
        ) as yp, tc.tile_pool(
            name="psS", bufs=2, space="PSUM"
        ) as psS, tc.tile_pool(
            name="xw", bufs=2
        ) as xw:
            qt_sb = pp.tile([128, 2, S], bf16)             # Q^T head pairs
            ktp_sb = pp.tile([128, HG, S], bf16)           # K^T padded/head
            v_sb = pp.tile([128, ST128, HG, VW], bf16)     # V | ones col
            ot_sb = pp.tile([128, 2, S], bf16)             # attn out^T
            wo_sb = pp.tile([128, 2, D], bf16)
            bias_sb = pp.tile([128, nb_alloc, 128], bf16)
            ones_sb = pp.tile([128, 128], bf16)            # bc stationary
            recb_sb = pp.tile([128, HG, QS], bf16)         # 1/den rows (@64)

            nc.gpsimd.memset(ones_sb[:], 1.0)
            nc.gpsimd.memset(
                recb_sb[:].rearrange("p a b -> p (a b)"), 0.0
            )
            nc.vector.memset(
                ktp_sb[:].rearrange("p a b -> p (a b)"), 0.0
            )
            nc.vector.memset(
                v_sb[:, :, :, 64:65].rearrange("p a b c -> p (a b c)"), 1.0
            )

            prefetched = {}

            def emit_scores_exp(qs, kt):
                kts, c0_min, sub_all = strips[qs]
                sub = sub_all[kt]
                nz = np.nonzero(sub)[0]
                c0 = int(nz.min()) * 128
                c1 = (int(nz.max()) + 1) * 128
                partial_js = [j for j in range(4) if sub[j] == 2]
                interior = [
                    j for j in range(4)
                    if sub[j] == 0 and c0 // 128 < j < c1 // 128
                ]
                pair_ps = {}
                for mt in range(2):
                    ps = psS.tile(
                        [128, 2, QS], f32, tag="ps", name=f"pp{mt}"
                    )
                    pair_ps[mt] = ps
                    for hh in range(2):
                        h = 2 * mt + hh
                        nc.tensor.matmul(
                            ps[:, hh, c0:c1],
                            ktp_sb[:, h, kt * 128 : (kt + 1) * 128],
                            qt_sb[:, mt, qs * QS + c0 : qs * QS + c1],
                            start=True,
                            stop=True,
                        )
                pair_p = {}
                for mt in range(2):
                    p_sb = pb.tile(
                        [128, 2, QS], bf16, tag="p", name=f"p{mt}"
                    )
                    pair_p[mt] = p_sb
                    for j in interior:
                        for hh in range(2):
                            nc.vector.memset(
                                p_sb[:, hh, j * 128 : (j + 1) * 128], 0.0
                            )
                    nc.scalar.activation(
                        p_sb[:, :, c0:c1],
                        pair_ps[mt][:, :, c0:c1],
                        EXP,
                        scale=0.125,
                    )
                    for j in partial_js:
                        bi = int(bidx[kt, 4 * qs + j])
                        for hh in range(2):
                            nc.vector.tensor_mul(
                                p_sb[:, hh, j * 128 : (j + 1) * 128],
                                p_sb[:, hh, j * 128 : (j + 1) * 128],
                                bias_sb[:, bi, :],
                            )
                return pair_p, c0

            # scores+exp of the first strips are emitted inside phase A,
            # interleaved with the projection chains: the ACT engine (the
            # strip phase pacer) starts its exp stream ~30us early.
            pf_units = [(0, kt) for kt in strips[0][0]]
            pf_units += [(1, kt) for kt in strips[1][0]][:N_PF1]
            pf_it = iter(list(pf_units))

            def pump_units(n):
                for _ in range(n):
                    u = next(pf_it, None)
                    if u is None:
                        return
                    prefetched[u] = emit_scores_exp(*u)

            # ---- Phase A: warmup + projections + prefetch ----
            deferred_v = []
            with tc.tile_pool(name="psA", bufs=4, space="PSUM") as psA:
                # Dummy matmuls: keep the PE active (and the HAM clock
                # un-throttled) while the first inputs stream in. They read
                # qt_sb before anything writes it — garbage values into a
                # scratch PSUM slot that is never read, zero dependencies.
                wps = psS.tile([128, 2, QS], f32, tag="ps", name="warm")
                for _ in range(N_WARM):
                    nc.tensor.matmul(
                        wps[:, 0, :], qt_sb[:, 0, 0:128], qt_sb[:, 0, 0:512],
                        start=True, stop=True,
                    )
                for x_d, w_d, which in (
                    (xq_d, wq_d, "q"),
                    (xk_d, wk_d, "k"),
                    (xv_d, wv_d, "v"),
                ):
                    xt = xw.tile([128, NQS, KT, QS], bf16, tag="xT")
                    wt = xw.tile([128, KT, DHG], bf16, tag="wT")
                    nc.sync.dma_start(out=wt[:], in_=w_d)
                    for qs in range(NQS):
                        nc.sync.dma_start(out=xt[:, qs], in_=x_d[qs])
                    if which == "q":
                        # queued behind the q inputs: needed only from the
                        # first strip's exp / y-projection onward
                        nc.sync.dma_start(out=wo_sb[:], in_=wo_d)
                        if n_bias:
                            nc.sync.dma_start(out=bias_sb[:], in_=bias_d)
                    if which in ("q", "k"):
                        for qs in range(NQS):
                            for mt in range(2):
                                ps = psA.tile([128, QS], f32, tag="pA")
                                for kt in range(KT):
                                    nc.tensor.matmul(
                                        ps[:],
                                        wt[:, kt, mt * 128 : (mt + 1) * 128],
                                        xt[:, qs, kt, :],
                                        start=(kt == 0),
                                        stop=(kt == KT - 1),
                                    )
                                if which == "q":
                                    nc.vector.tensor_copy(
                                        out=qt_sb[
                                            :, mt, qs * QS : (qs + 1) * QS
                                        ],
                                        in_=ps[:],
                                    )
                                else:
                                    # per-head K-padded keys: head at its
                                    # own partition range, others stay zero
                                    for hh in range(2):
                                        h = 2 * mt + hh
                                        po = 64 * hh
                                        nc.vector.tensor_copy(
                                            out=ktp_sb[
                                                po : po + 64, h,
                                                qs * QS : (qs + 1) * QS,
                                            ],
                                            in_=ps[po : po + 64, :],
                                        )
                            if which == "k" and qs <= 1:
                                pump_units(2)
                    else:
                        def make_vchain(st, xt=xt, wt=wt):
                            # deferred V chain: runs from a psS slot during
                            # the s0->s1 normalize stall — the only strip
                            # needing these key tiles is the last one
                            def f():
                                qv, sc = divmod(st, NQS)
                                ps = psS.tile([128, 2, QS], f32, tag="ps",
                                              name=f"vd{st}")
                                for kt in range(KT):
                                    nc.tensor.matmul(
                                        ps[:, 0, :DHG],
                                        xt[:, qv, kt,
                                           sc * 128 : (sc + 1) * 128],
                                        wt[:, kt, :],
                                        start=(kt == 0),
                                        stop=(kt == KT - 1),
                                    )
                                nc.vector.tensor_copy(
                                    out=v_sb[:, st, :, 0:DK],
                                    in_=ps[:, 0, :DHG].rearrange(
                                        "p (h d) -> p h d", h=HG
                                    ),
                                )
                            return f

                        for st in range(ST128):
                            if st >= ST128 - 2:
                                deferred_v.append(make_vchain(st))
                                continue
                            qs, sc = divmod(st, NQS)
                            ps = psA.tile([128, QS], f32, tag="pA")
                            for kt in range(KT):
                                nc.tensor.matmul(
                                    ps[:, :DHG],
                                    xt[:, qs, kt, sc * 128 : (sc + 1) * 128],
                                    wt[:, kt, :],
                                    start=(kt == 0),
                                    stop=(kt == KT - 1),
                                )
                            nc.vector.tensor_copy(
                                out=v_sb[:, st, :, 0:DK],
                                in_=ps[:, :DHG].rearrange(
                                    "p (h d) -> p h d", h=HG
                                ),
                            )
                            pump_units(1)
                pump_units(len(pf_units))

            # ---- Phases B+C interleaved per q-strip ----
            # recip-gated boundary instructions get LOW priority (large
            # value) so the list scheduler statically orders any ready
            # filler work ahead of them; dependency edges still pin their
            # execution, so this is order-only, correctness-neutral
            deprio = [1 << 20]

            def lower(mm):
                mm.bass_priority = deprio[0]
                deprio[0] += 1

            with tc.tile_pool(name="psOT", bufs=1, space="PSUM") as psOT:

                def emit_yproj(st, hi=False):
                    ps = psS.tile(
                        [128, 2, QS], f32, tag="ps", name=f"py{st}"
                    )
                    # hi: boundary-filler y-projections — schedule their
                    # matmuls ahead of the recip-gated bc/PV ops (the psS
                    # ring still gates them late enough not to hoist)
                    ctx = tc.high_priority() if hi else nullcontext()
                    with ctx:
                        for nh in range(2):
                            for mt in range(2):
                                nc.tensor.matmul(
                                    ps[:, nh, :],
                                    ot_sb[:, mt, st * 128 : (st + 1) * 128],
                                    wo_sb[:, mt, nh * QS : (nh + 1) * QS],
                                    start=(mt == 0),
                                    stop=(mt == 1),
                                )
                    y_sb = yp.tile([128, 2, QS], bf16, tag="y",
                                   name=f"ysb{st}")
                    nc.vector.tensor_copy(
                        out=y_sb[:].rearrange("p a b -> p (a b)"),
                        in_=ps[:].rearrange("p a b -> p (a b)"),
                    )
                    nc.sync.dma_start(
                        out=y_d[st * 128 : (st + 1) * 128, :],
                        in_=y_sb[:].rearrange("p a b -> p (a b)"),
                    )

                norm_pending = None
                pump_idx = {1: N_PF1}
                for qs in range(NQS):
                    kts, c0_min, sub_all = strips[qs]
                    pend_y = (
                        [(qs - 1) * (QS // 128) + i for i in range(QS // 128)]
                        if qs
                        else []
                    )

                    def alloc_pots():
                        # one tile, 4 banks: head h in column block h, its
                        # softmax denominator on partition row 64 — so the
                        # 4 denominator rows form a single AP and the strip
                        # reciprocal is ONE unsplittable ACT instruction
                        pots = psOT.tile(
                            [128, HG, QS], f32, tag="pot", name="pots"
                        )
                        if c0_min > 0:
                            nc.vector.memset(
                                pots[0:65, :, :].rearrange(
                                    "p a b -> p (a b)"
                                ),
                                0.0,
                            )
                        return pots

                    pots = None if norm_pending else alloc_pots()
                    deferred_pv = []
                    for idx, kt in enumerate(kts):
                        was_pf = (qs, kt) in prefetched
                        if was_pf:
                            pair_p, c0 = prefetched.pop((qs, kt))
                        else:
                            pair_p, c0 = emit_scores_exp(qs, kt)
                        # cascade: while this strip rides on prefetched
                        # exps (ACT slack), pre-emit the next strip's first
                        # score/exp units so its boundary hides the same way
                        if was_pf and qs + 1 < NQS:
                            nkts = strips[qs + 1][0]
                            j = pump_idx.get(qs + 1, 0)
                            if j < min(len(nkts), N_PF1):
                                pump_idx[qs + 1] = j + 1
                                prefetched[(qs + 1, nkts[j])] = (
                                    emit_scores_exp(qs + 1, nkts[j])
                                )
                        if norm_pending is not None and idx == 1:
                            # ungated PE filler for the normalize funnel:
                            # the deferred V chains depend only on resident
                            # phase-A data
                            while deferred_v:
                                deferred_v.pop(0)()
                            norm_pending()
                            norm_pending = None
                            pots = alloc_pots()

                        def emit_pv(lo=False, kt=kt, idx=idx, c0=c0,
                                    pair_p=pair_p):
                            for h in range(HG):
                                mm = nc.tensor.matmul(
                                    pots[0:65, h, c0:],
                                    v_sb[:, kt, h, 0:65],
                                    pair_p[h // 2][:, h % 2, c0:],
                                    start=(idx == 0 and c0_min == 0),
                                    stop=(idx == len(kts) - 1),
                                    skip_group_check=(c0_min > 0),
                                )
                                if lo:
                                    lower(mm)

                        if pots is None:
                            deferred_pv.append(emit_pv)
                        else:
                            while deferred_pv:
                                deferred_pv.pop(0)(True)
                            emit_pv()
                        # hold the last 2 back: they read the PREVIOUS
                        # strip's (long-finished) output, so they are
                        # ungated filler for the strip-end normalize funnel
                        if len(pend_y) > 2 and idx >= 3:
                            emit_yproj(pend_y.pop(0))

                    def make_norm(qs=qs, pots=pots):
                        def norm():
                            # one recip(ACT) + PE ones-broadcast + DVE scale
                            otu = otup.tile([128, HG, QS], f32, tag="otu")
                            with tc.high_priority():
                                raw_act(
                                    nc, recb_sb[64:65, :, :],
                                    pots[64:65, :, :], RECIP,
                                )
                                nc.vector.tensor_copy(
                                    out=otu[0:64, :, :].rearrange(
                                        "p a b -> p (a b)"
                                    ),
                                    in_=pots[0:64, :, :].rearrange(
                                        "p a b -> p (a b)"
                                    ),
                                )
                            bct = {}
                            for h in range(HG):
                                po = 64 * (h % 2)
                                mt = h // 2
                                hh = h % 2
                                if hh == 0:
                                    bct[mt] = psS.tile(
                                        [128, 2, QS], f32, tag="ps",
                                        name=f"bc{mt}",
                                    )
                                lower(nc.tensor.matmul(
                                    bct[mt][:, hh, :],
                                    ones_sb[:],
                                    recb_sb[:, h, :],
                                    start=True,
                                    stop=True,
                                ))
                                nc.vector.tensor_mul(
                                    ot_sb[
                                        po : po + 64, mt,
                                        qs * QS : (qs + 1) * QS,
                                    ],
                                    otu[0:64, h, :],
                                    bct[mt][0:64, hh, :],
                                )
                        return norm

                    for st in pend_y:
                        emit_yproj(st, hi=True)
                    norm_pending = make_norm()
                    if qs == NQS - 1:
                        norm_pending()
                        norm_pending = None
                        for sti in range(QS // 128):
                            emit_yproj(qs * (QS // 128) + sti)

    _split_waits(nc)
    return nc


_program_cache = {}


def get_program(cls, bidx, n_bias):
    key = (cls.tobytes(), bidx.tobytes(), n_bias)
    if key not in _program_cache:
        _program_cache[key] = build_program(cls, bidx, n_bias)
    return _program_cache[key]


def _x_layout(xb):
    """x[b].T as [NQS, 128, KT, QS] bf16 (strip-major, partition-major)."""
    xt = np.asarray(xb, np.float32).T                       # [D, S]
    arr = xt.reshape(KT, 128, NQS, QS).transpose(2, 1, 0, 3)
    return np.ascontiguousarray(arr).astype(ml_dtypes.bfloat16)


def _w_layout(w, rows):
    """w[rows].T as [128, KT, DHG] bf16."""
    wt = np.asarray(w, np.float32)[rows].T                  # [D, DHG]
    arr = wt.reshape(KT, 128, DHG).transpose(1, 0, 2)
    return np.ascontiguousarray(arr).astype(ml_dtypes.bfloat16)


def _wo_layout(w_o, rows):
    """w_o[:, rows].T as [128, 2, D] bf16."""
    wt = np.asarray(w_o, np.float32)[:, rows].T             # [DHG, D]
    arr = wt.reshape(2, 128, D).transpose(1, 0, 2)
    return np.ascontiguousarray(arr).astype(ml_dtypes.bfloat16)


def make_in_maps(q, k, v, mask, w_q, w_k, w_v, w_o, biases):
    if biases:
        bias_arr = np.ascontiguousarray(
            np.stack(biases).transpose(1, 0, 2)
        ).astype(ml_dtypes.bfloat16)
    else:
        bias_arr = np.zeros((128, 1, 128), ml_dtypes.bfloat16)
    xs = [
        {"xqT": _x_layout(q[b]), "xkT": _x_layout(k[b]),
         "xvT": _x_layout(v[b])}
        for b in range(B)
    ]
    ws = []
    for g in range(4):
        rows = slice(g * DHG, (g + 1) * DHG)
        ws.append(
            {
                "wqT": _w_layout(w_q, rows),
                "wkT": _w_layout(w_k, rows),
                "wvT": _w_layout(w_v, rows),
                "woT": _wo_layout(w_o, rows),
            }
        )
    in_maps = []
    for c in range(NCORES):
        b, g = divmod(c, 4)
        in_maps.append({**xs[b], **ws[g], "biasT": bias_arr})
    return in_maps


def combine_results(results):
    out = np.empty((B, S, D), np.float32)
    for b in range(B):
        acc = results[4 * b]["y"].astype(np.float32)
        for g in range(1, 4):
            acc = acc + results[4 * b + g]["y"].astype(np.float32)
        out[b] = acc
    return out


def kernel(q, k, v, mask, w_q, w_k, w_v, w_o):
    q = np.asarray(q, np.float32)
    k = np.asarray(k, np.float32)
    v = np.asarray(v, np.float32)
    w_q = np.asarray(w_q, np.float32)
    w_k = np.asarray(w_k, np.float32)
    w_v = np.asarray(w_v, np.float32)
    w_o = np.asarray(w_o, np.float32)
    maskT = np.ascontiguousarray(
        np.broadcast_to(np.asarray(mask), (1, 1, S, S))[0, 0].T
    )
    cls, bidx, biases = classify_mask(maskT)
    nc = get_program(cls, bidx, len(biases))
    in_maps = make_in_maps(q, k, v, mask, w_q, w_k, w_v, w_o, biases)
    res = run_bass_kernel_spmd(nc, in_maps, list(range(NCORES)))
    return combine_results(res.results)


# revision 56
# speedup vs baseline: 1.0240x; 1.0031x over previous
"""Multi-head attention (B=2, S=2048, D=1024, H=16) on 8 trn2 NeuronCores.

Sharding: batch (2) x head-groups (4 heads each, 4 groups) = 8 cores.
Each core computes Q/K/V projections for its 4 heads on its batch,
causal-masked softmax attention, and a partial output projection
(row-sharded w_o); the host sums the 4 partials per batch.

Layout strategy: the host stages every DRAM input in the exact SBUF
layout (partition-major, strip-major x) so each DMA is 128 fully
contiguous descriptors. All matmul contractions run over the SBUF
partition axis with no on-device transposes. Attention scores are
computed transposed (ST[k, q]) so P = exp(ST) feeds the PV matmul
directly, and V carries an appended ones-column (M=65 stationary)
whose output row is the softmax denominator.

v2 scheduling: a burst of dummy matmuls at t=0 holds the PE's HAM
clock-gate open through the initial input DMA; softmax normalization
uses ACT evacuation + reciprocal_approx_fast + a GpSimd partition
broadcast (instead of serial DVE reciprocal/stream-shuffle chains) so
strip boundaries no longer stall the PE; PV accumulation orders the
widest causal window first so no PSUM memsets are needed.
"""
import sys
from contextlib import nullcontext

sys.path.insert(0, "/opt/trn_rl_repo")

import numpy as np
import ml_dtypes

import concourse.bass as bass
import concourse.mybir as mybir
import concourse.tile as tile
from concourse.bass_utils import run_bass_kernel_spmd

B, S, D, H, DK = 2, 2048, 1024, 16, 64
NCORES = 8
HG = 4                # heads per core
DHG = HG * DK         # 256 head-dims per core
KT = D // 128         # 8 contraction tiles for the projections
ST128 = S // 128      # 16 128-row tiles of S
QS = 512              # q-strip width
NQS = S // QS         # 4 strips
VW = 68               # v_sb inner stride (65 used, padded for alignment)
N_WARM = 16           # dummy matmuls to hold the HAM clock-gate open
N_PF1 = 8             # strip-1 score/exp units prefetched into phase A
PB_BUFS = 26          # p-tile ring (holds the prefetched exp outputs)

f32 = mybir.dt.float32
bf16 = mybir.dt.bfloat16
EXP = mybir.ActivationFunctionType.Exp
RECIP = mybir.ActivationFunctionType.Reciprocal


def raw_act(nc, out, in_, func, bias=0.0, scale=1.0):
    """InstActivation without the bass wrapper. Needed for Reciprocal,
    which the wrapper refuses; measured on this hardware it is ~1e-5
    relative error over the softmax-denominator range, far inside our
    tolerance, and one ACT op replaces a 3.3us single-lane DVE
    reciprocal on the strip-end critical path."""
    eng = nc.scalar
    inputs = [
        eng.lower_ap(in_),
        mybir.ImmediateValue(dtype=f32, value=bias),
        mybir.ImmediateValue(dtype=f32, value=scale),
        mybir.ImmediateValue(dtype=f32, value=0.0),
    ]
    return eng.add_instruction(
        mybir.InstActivation(
            name=nc.get_next_instruction_name(),
            func=func,
            ins=inputs,
            outs=[eng.lower_ap(out)],
        )
    )


def _split_waits(nc, max_waits=1):
    """This walrus build rejects >1 SyncWait per instruction (and >0 on
    fp32-family matmuls, which lower through the 1-wait S3_LW struct).
    Hoist excess waits onto dedicated NOPs on the same engine queue."""
    n = 0
    for fn in nc.m.functions:
        for blk in fn.blocks:
            new = []
            for ins in blk.instructions:
                si = getattr(ins, "sync_info", None)
                if si is not None and si.on_wait:
                    limit = 0 if isinstance(ins, mybir.InstMatmult) else max_waits
                    if len(si.on_wait) > limit:
                        waits = list(si.on_wait)
                        hoist = waits if limit == 0 else waits[:-limit]
                        keep = [] if limit == 0 else waits[-limit:]
                        for w in hoist:
                            n += 1
                            new.append(
                                mybir.InstNoOp(
                                    name=f"I-waitfix-{n}",
                                    engine=ins.engine,
                                    bass_nofuse=True,
                                    sync_info=mybir.SyncInfo(
                                        on_wait=[w], on_update=[]
                                    ),
                                )
                            )
                        ins.sync_info = mybir.SyncInfo(
                            on_wait=keep, on_update=list(si.on_update)
                        )
                new.append(ins)
            blk.instructions[:] = new
    return n


def classify_mask(maskT):
    """Block-classify the transposed mask at 128x128 granularity.
    Returns (cls[i,j] in {0 empty,1 full,2 partial}, bias index map,
    list of multiplicative {0,1} blocks for the partial ones)."""
    nb = S // 128
    cls = np.empty((nb, nb), dtype=np.int8)
    bidx = np.full((nb, nb), -1, dtype=np.int32)
    biases = []
    for i in range(nb):
        for j in range(nb):
            blk = maskT[i * 128 : (i + 1) * 128, j * 128 : (j + 1) * 128]
            if (blk != 0).all():
                cls[i, j] = 1
            elif (blk == 0).all():
                cls[i, j] = 0
            else:
                cls[i, j] = 2
                bidx[i, j] = len(biases)
                biases.append(
                    (blk != 0).astype(np.float32)
                )
    return cls, bidx, biases


def build_program(cls, bidx, n_bias):
    nb_alloc = max(1, n_bias)
    nc = bass.Bass("TRN2", target_bir_lowering=False, debug=False,
                   num_devices=NCORES)
    xq_d = nc.dram_tensor("xqT", [NQS, 128, KT, QS], bf16,
                          kind="ExternalInput").ap()
    xk_d = nc.dram_tensor("xkT", [NQS, 128, KT, QS], bf16,
                          kind="ExternalInput").ap()
    xv_d = nc.dram_tensor("xvT", [NQS, 128, KT, QS], bf16,
                          kind="ExternalInput").ap()
    wq_d = nc.dram_tensor("wqT", [128, KT, DHG], bf16,
                          kind="ExternalInput").ap()
    wk_d = nc.dram_tensor("wkT", [128, KT, DHG], bf16,
                          kind="ExternalInput").ap()
    wv_d = nc.dram_tensor("wvT", [128, KT, DHG], bf16,
                          kind="ExternalInput").ap()
    wo_d = nc.dram_tensor("woT", [128, 2, D], bf16, kind="ExternalInput").ap()
    bias_d = nc.dram_tensor("biasT", [128, nb_alloc, 128], bf16,
                            kind="ExternalInput").ap()
    y_d = nc.dram_tensor("y", [S, D], bf16, kind="ExternalOutput").ap()

    # per-strip causal-window metadata, shared by the prefetch and the
    # strip loop. kts order: a widest-window tile first (its PV opens the
    # PSUM accumulation and must cover every later window), then partial
    # (masked) tiles so their DVE mask-multiplies run early, then the rest.
    strips = []
    for qs in range(NQS):
        sub_all = cls[:, 4 * qs : 4 * qs + 4]
        kts = [i for i in range(ST128) if sub_all[i].any()]

        def window(kt):
            nz = np.nonzero(sub_all[kt])[0]
            return int(nz.min()) * 128

        c0_min = min((window(kt) for kt in kts), default=0)
        parts = sorted(
            (kt for kt in kts if (sub_all[kt] == 2).any()), key=window
        )
        fulls = sorted(
            (kt for kt in kts if (sub_all[kt] != 2).all()), key=window
        )
        cand = [kt for kt in fulls if window(kt) == c0_min] or [
            kt for kt in parts if window(kt) == c0_min
        ]
        kts = [cand[0]] + [kt for kt in parts + fulls if kt != cand[0]]
        strips.append((kts, c0_min, sub_all))

    with tile.TileContext(nc) as tc:
        with tc.tile_pool(name="persist", bufs=1) as pp, tc.tile_pool(
            name="pb", bufs=PB_BUFS
        ) as pb, tc.tile_pool(
            name="otu", bufs=2
        ) as otup, tc.tile_pool(
            name="yp", bufs=3
        ) as yp, tc.tile_pool(
            name="psS", bufs=2, space="PSUM"
        ) as psS, tc.tile_pool(
            name="xw", bufs=2
        ) as xw:
            qt_sb = pp.tile([128, 2, S], bf16)             # Q^T head pairs
            ktp_sb = pp.tile([128, HG, S], bf16)           # K^T padded/head
            v_sb = pp.tile([128, ST128, HG, VW], bf16)     # V | ones col
            ot_sb = pp.tile([128, 2, S], bf16)             # attn out^T
            wo_sb = pp.tile([128, 2, D], bf16)
            bias_sb = pp.tile([128, nb_alloc, 128], bf16)
            ones_sb = pp.tile([128, 128], bf16)            # bc stationary
            recb_sb = pp.tile([128, HG, QS], bf16)         # 1/den rows (@64)

            nc.gpsimd.memset(ones_sb[:], 1.0)
            nc.gpsimd.memset(
                recb_sb[:].rearrange("p a b -> p (a b)"), 0.0
            )
            nc.vector.memset(
                ktp_sb[:].rearrange("p a b -> p (a b)"), 0.0
            )
            nc.vector.memset(
                v_sb[:, :, :, 64:65].rearrange("p a b c -> p (a b c)"), 1.0
            )

            prefetched = {}

            def emit_scores_exp(qs, kt):
                kts, c0_min, sub_all = strips[qs]
                sub = sub_all[kt]
                nz = np.nonzero(sub)[0]
                c0 = int(nz.min()) * 128
                c1 = (int(nz.max()) + 1) * 128
                partial_js = [j for j in range(4) if sub[j] == 2]
                interior = [
                    j for j in range(4)
                    if sub[j] == 0 and c0 // 128 < j < c1 // 128
                ]
                pair_ps = {}
                for mt in range(2):
                    ps = psS.tile(
                        [128, 2, QS], f32, tag="ps", name=f"pp{mt}"
                    )
                    pair_ps[mt] = ps
                    for hh in range(2):
                        h = 2 * mt + hh
                        nc.tensor.matmul(
                            ps[:, hh, c0:c1],
                            ktp_sb[:, h, kt * 128 : (kt + 1) * 128],
                            qt_sb[:, mt, qs * QS + c0 : qs * QS + c1],
                            start=True,
                            stop=True,
                        )
                pair_p = {}
                for mt in range(2):
                    p_sb = pb.tile(
                        [128, 2, QS], bf16, tag="p", name=f"p{mt}"
                    )
                    pair_p[mt] = p_sb
                    for j in interior:
                        for hh in range(2):
                            nc.vector.memset(
                                p_sb[:, hh, j * 128 : (j + 1) * 128], 0.0
                            )
                    nc.scalar.activation(
                        p_sb[:, :, c0:c1],
                        pair_ps[mt][:, :, c0:c1],
                        EXP,
                        scale=0.125,
                    )
                    for j in partial_js:
                        bi = int(bidx[kt, 4 * qs + j])
                        for hh in range(2):
                            nc.vector.tensor_mul(
                                p_sb[:, hh, j * 128 : (j + 1) * 128],
                                p_sb[:, hh, j * 128 : (j + 1) * 128],
                                bias_sb[:, bi, :],
                            )
                return pair_p, c0

            # scores+exp of the first strips are emitted inside phase A,
            # interleaved with the projection chains: the ACT engine (the
            # strip phase pacer) starts its exp stream ~30us early.
            pf_units = [(0, kt) for kt in strips[0][0]]
            pf_units += [(1, kt) for kt in strips[1][0]][:N_PF1]
            pf_it = iter(list(pf_units))

            def pump_units(n):
                for _ in range(n):
                    u = next(pf_it, None)
                    if u is None:
                        return
                    prefetched[u] = emit_scores_exp(*u)

            # ---- Phase A: warmup + projections + prefetch ----
            deferred_v = []
            with tc.tile_pool(name="psA", bufs=4, space="PSUM") as psA:
                # Dummy matmuls: keep the PE active (and the HAM clock
                # un-throttled) while the first inputs stream in. They read
                # qt_sb before anything writes it — garbage values into a
                # scratch PSUM slot that is never read, zero dependencies.
                wps = psS.tile([128, 2, QS], f32, tag="ps", name="warm")
                for _ in range(N_WARM):
                    nc.tensor.matmul(
                        wps[:, 0, :], qt_sb[:, 0, 0:128], qt_sb[:, 0, 0:512],
                        start=True, stop=True,
                    )
                for x_d, w_d, which in (
                    (xq_d, wq_d, "q"),
                    (xk_d, wk_d, "k"),
                    (xv_d, wv_d, "v"),
                ):
                    xt = xw.tile([128, NQS, KT, QS], bf16, tag="xT")
                    wt = xw.tile([128, KT, DHG], bf16, tag="wT")
                    nc.sync.dma_start(out=wt[:], in_=w_d)
                    for qs in range(NQS):
                        nc.sync.dma_start(out=xt[:, qs], in_=x_d[qs])
                    if which == "q":
                        # queued behind the q inputs: needed only from the
                        # first strip's exp / y-projection onward
                        nc.sync.dma_start(out=wo_sb[:], in_=wo_d)
                        if n_bias:
                            nc.sync.dma_start(out=bias_sb[:], in_=bias_d)
                    if which in ("q", "k"):
                        for qs in range(NQS):
                            for mt in range(2):
                                ps = psA.tile([128, QS], f32, tag="pA")
                                for kt in range(KT):
                                    nc.tensor.matmul(
                                        ps[:],
                                        wt[:, kt, mt * 128 : (mt + 1) * 128],
                                        xt[:, qs, kt, :],
                                        start=(kt == 0),
                                        stop=(kt == KT - 1),
                                    )
                                if which == "q":
                                    nc.vector.tensor_copy(
                                        out=qt_sb[
                                            :, mt, qs * QS : (qs + 1) * QS
                                        ],
                                        in_=ps[:],
                                    )
                                else:
                                    # per-head K-padded keys: head at its
                                    # own partition range, others stay zero
                                    for hh in range(2):
                                        h = 2 * mt + hh
                                        po = 64 * hh
                                        nc.vector.tensor_copy(
                                            out=ktp_sb[
                                                po : po + 64, h,
                                                qs * QS : (qs + 1) * QS,
                                            ],
                                            in_=ps[po : po + 64, :],
                                        )
                            if which == "k" and qs <= 1:
                                pump_units(2)
                    else:
                        def make_vchain(st, xt=xt, wt=wt):
                            # deferred V chain: runs from a psS slot during
                            # the s0->s1 normalize stall — the only strip
                            # needing these key tiles is the last one
                            def f():
                                qv, sc = divmod(st, NQS)
                                ps = psS.tile([128, 2, QS], f32, tag="ps",
                                              name=f"vd{st}")
                                for kt in range(KT):
                                    nc.tensor.matmul(
                                        ps[:, 0, :DHG],
                                        xt[:, qv, kt,
                                           sc * 128 : (sc + 1) * 128],
                                        wt[:, kt, :],
                                        start=(kt == 0),
                                        stop=(kt == KT - 1),
                                    )
                                nc.vector.tensor_copy(
                                    out=v_sb[:, st, :, 0:DK],
                                    in_=ps[:, 0, :DHG].rearrange(
                                        "p (h d) -> p h d", h=HG
                                    ),
                                )
                            return f

                        for st in range(ST128):
                            if st >= ST128 - 2:
                                deferred_v.append(make_vchain(st))
                                continue
                            qs, sc = divmod(st, NQS)
                            ps = psA.tile([128, QS], f32, tag="pA")
                            for kt in range(KT):
                                nc.tensor.matmul(
                                    ps[:, :DHG],
                                    xt[:, qs, kt, sc * 128 : (sc + 1) * 128],
                                    wt[:, kt, :],
                                    start=(kt == 0),
                                    stop=(kt == KT - 1),
                                )
                            nc.vector.tensor_copy(
                                out=v_sb[:, st, :, 0:DK],
                                in_=ps[:, :DHG].rearrange(
                                    "p (h d) -> p h d", h=HG
                                ),
                            )
                            pump_units(1)
                pump_units(len(pf_units))

            # ---- Phases B+C interleaved per q-strip ----
            # recip-gated boundary instructions get LOW priority (large
            # value) so the list scheduler statically orders any ready
            # filler work ahead of them; dependency edges still pin their
            # execution, so this is order-only, correctness-neutral
            deprio = [1 << 20]

            def lower(mm):
                mm.bass_priority = deprio[0]
                deprio[0] += 1

            with tc.tile_pool(name="psOT", bufs=1, space="PSUM") as psOT:

                def emit_yproj(st, hi=False):
                    ps = psS.tile(
                        [128, 2, QS], f32, tag="ps", name=f"py{st}"
                    )
                    # hi: boundary-filler y-projections — schedule their
                    # matmuls ahead of the recip-gated bc/PV ops (the psS
                    # ring still gates them late enough not to hoist)
                    ctx = tc.high_priority() if hi else nullcontext()
                    with ctx:
                        for nh in range(2):
                            for mt in range(2):
                                nc.tensor.matmul(
                                    ps[:, nh, :],
                                    ot_sb[:, mt, st * 128 : (st + 1) * 128],
                                    wo_sb[:, mt, nh * QS : (nh + 1) * QS],
                                    start=(mt == 0),
                                    stop=(mt == 1),
                                )
                    y_sb = yp.tile([128, 2, QS], bf16, tag="y",
                                   name=f"ysb{st}")
                    nc.vector.tensor_copy(
                        out=y_sb[:].rearrange("p a b -> p (a b)"),
                        in_=ps[:].rearrange("p a b -> p (a b)"),
                    )
                    nc.sync.dma_start(
                        out=y_d[st * 128 : (st + 1) * 128, :],
                        in_=y_sb[:].rearrange("p a b -> p (a b)"),
                    )

                norm_pending = None
                pump_idx = {1: N_PF1}
                for qs in range(NQS):
                    kts, c0_min, sub_all = strips[qs]
                    pend_y = (
                        [(qs - 1) * (QS // 128) + i for i in range(QS // 128)]
                        if qs
                        else []
                    )

                    def alloc_pots():
                        # one tile, 4 banks: head h in column block h, its
                        # softmax denominator on partition row 64 — so the
                        # 4 denominator rows form a single AP and the strip
                        # reciprocal is ONE unsplittable ACT instruction
                        pots = psOT.tile(
                            [128, HG, QS], f32, tag="pot", name="pots"
                        )
                        if c0_min > 0:
                            nc.vector.memset(
                                pots[0:65, :, :].rearrange(
                                    "p a b -> p (a b)"
                                ),
                                0.0,
                            )
                        return pots

                    pots = None if norm_pending else alloc_pots()
                    deferred_pv = []
                    for idx, kt in enumerate(kts):
                        was_pf = (qs, kt) in prefetched
                        if was_pf:
                            pair_p, c0 = prefetched.pop((qs, kt))
                        else:
                            pair_p, c0 = emit_scores_exp(qs, kt)
                        # cascade: while this strip rides on prefetched
                        # exps (ACT slack), pre-emit the next strip's first
                        # score/exp units so its boundary hides the same way
                        if was_pf and qs + 1 < NQS:
                            nkts = strips[qs + 1][0]
                            j = pump_idx.get(qs + 1, 0)
                            if j < min(len(nkts), N_PF1):
                                pump_idx[qs + 1] = j + 1
                                prefetched[(qs + 1, nkts[j])] = (
                                    emit_scores_exp(qs + 1, nkts[j])
                                )
                        if norm_pending is not None and idx == 1:
                            # ungated PE filler for the normalize funnel:
                            # the deferred V chains depend only on resident
                            # phase-A data
                            while deferred_v:
                                deferred_v.pop(0)()
                            norm_pending()
                            norm_pending = None
                            pots = alloc_pots()

                        def emit_pv(lo=False, kt=kt, idx=idx, c0=c0,
                                    pair_p=pair_p):
                            for h in range(HG):
                                mm = nc.tensor.matmul(
                                    pots[0:65, h, c0:],
                                    v_sb[:, kt, h, 0:65],
                                    pair_p[h // 2][:, h % 2, c0:],
                                    start=(idx == 0 and c0_min == 0),
                                    stop=(idx == len(kts) - 1),
                                    skip_group_check=(c0_min > 0),
                                )
                                if lo:
                                    lower(mm)

                        if pots is None:
                            deferred_pv.append(emit_pv)
                        else:
                            with tc.tile_wait_until(0.001 * (qs + 1)):
                                while deferred_pv:
                                    deferred_pv.pop(0)(True)
                            emit_pv()
                        # hold the last 2 back: they read the PREVIOUS
                        # strip's (long-finished) output, so they are
                        # ungated filler for the strip-end normalize funnel
                        if len(pend_y) > 2 and idx >= 3:
                            emit_yproj(pend_y.pop(0))

                    def make_norm(qs=qs, pots=pots):
                        def norm():
                            # one recip(ACT) + PE ones-broadcast + DVE scale
                            otu = otup.tile([128, HG, QS], f32, tag="otu")
                            with tc.high_priority():
                                raw_act(
                                    nc, recb_sb[64:65, :, :],
                                    pots[64:65, :, :], RECIP,
                                )
                                nc.vector.tensor_copy(
                                    out=otu[0:64, :, :].rearrange(
                                        "p a b -> p (a b)"
                                    ),
                                    in_=pots[0:64, :, :].rearrange(
                                        "p a b -> p (a b)"
                                    ),
                                )
                            bct = {}
                            for h in range(HG):
                                po = 64 * (h % 2)
                                mt = h // 2
                                hh = h % 2
                                if hh == 0:
                                    bct[mt] = psS.tile(
                                        [128, 2, QS], f32, tag="ps",
                                        name=f"bc{mt}",
                                    )
                                with tc.tile_wait_until(0.001 * (qs + 1)):
                                    lower(nc.tensor.matmul(
                                        bct[mt][:, hh, :],
                                        ones_sb[:],
                                        recb_sb[:, h, :],
                                        start=True,
                                        stop=True,
                                    ))
                                nc.vector.tensor_mul(
                                    ot_sb[
                                        po : po + 64, mt,
                                        qs * QS : (qs + 1) * QS,
                                    ],
                                    otu[0:64, h, :],
                                    bct[mt][0:64, hh, :],
                                )
                        return norm

                    for st in pend_y:
                        emit_yproj(st, hi=True)
                    norm_pending = make_norm()
                    if qs == NQS - 1:
                        norm_pending()
                        norm_pending = None
                        for sti in range(QS // 128):
                            emit_yproj(qs * (QS // 128) + sti)

    _split_waits(nc)
    return nc


_program_cache = {}


def get_program(cls, bidx, n_bias):
    key = (cls.tobytes(), bidx.tobytes(), n_bias)
    if key not in _program_cache:
        _program_cache[key] = build_program(cls, bidx, n_bias)
    return _program_cache[key]


def _x_layout(xb):
    """x[b].T as [NQS, 128, KT, QS] bf16 (strip-major, partition-major)."""
    xt = np.asarray(xb, np.float32).T                       # [D, S]
    arr = xt.reshape(KT, 128, NQS, QS).transpose(2, 1, 0, 3)
    return np.ascontiguousarray(arr).astype(ml_dtypes.bfloat16)


def _w_layout(w, rows):
    """w[rows].T as [128, KT, DHG] bf16."""
    wt = np.asarray(w, np.float32)[rows].T                  # [D, DHG]
    arr = wt.reshape(KT, 128, DHG).transpose(1, 0, 2)
    return np.ascontiguousarray(arr).astype(ml_dtypes.bfloat16)


def _wo_layout(w_o, rows):
    """w_o[:, rows].T as [128, 2, D] bf16."""
    wt = np.asarray(w_o, np.float32)[:, rows].T             # [DHG, D]
    arr = wt.reshape(2, 128, D).transpose(1, 0, 2)
    return np.ascontiguousarray(arr).astype(ml_dtypes.bfloat16)


def make_in_maps(q, k, v, mask, w_q, w_k, w_v, w_o, biases):
    if biases:
        bias_arr = np.ascontiguousarray(
            np.stack(biases).transpose(1, 0, 2)
        ).astype(ml_dtypes.bfloat16)
    else:
        bias_arr = np.zeros((128, 1, 128), ml_dtypes.bfloat16)
    xs = [
        {"xqT": _x_layout(q[b]), "xkT": _x_layout(k[b]),
         "xvT": _x_layout(v[b])}
        for b in range(B)
    ]
    ws = []
    for g in range(4):
        rows = slice(g * DHG, (g + 1) * DHG)
        ws.append(
            {
                "wqT": _w_layout(w_q, rows),
                "wkT": _w_layout(w_k, rows),
                "wvT": _w_layout(w_v, rows),
                "woT": _wo_layout(w_o, rows),
            }
        )
    in_maps = []
    for c in range(NCORES):
        b, g = divmod(c, 4)
        in_maps.append({**xs[b], **ws[g], "biasT": bias_arr})
    return in_maps


def combine_results(results):
    out = np.empty((B, S, D), np.float32)
    for b in range(B):
        acc = results[4 * b]["y"].astype(np.float32)
        for g in range(1, 4):
            acc = acc + results[4 * b + g]["y"].astype(np.float32)
        out[b] = acc
    return out


def kernel(q, k, v, mask, w_q, w_k, w_v, w_o):
    q = np.asarray(q, np.float32)
    k = np.asarray(k, np.float32)
    v = np.asarray(v, np.float32)
    w_q = np.asarray(w_q, np.float32)
    w_k = np.asarray(w_k, np.float32)
    w_v = np.asarray(w_v, np.float32)
    w_o = np.asarray(w_o, np.float32)
    maskT = np.ascontiguousarray(
        np.broadcast_to(np.asarray(mask), (1, 1, S, S))[0, 0].T
    )
    cls, bidx, biases = classify_mask(maskT)
    nc = get_program(cls, bidx, len(biases))
    in_maps = make_in_maps(q, k, v, mask, w_q, w_k, w_v, w_o, biases)
    res = run_bass_kernel_spmd(nc, in_maps, list(range(NCORES)))
    return combine_results(res.results)
